# revision 3
# baseline (speedup 1.0000x reference)
"""3-layer GCN (gcn_norm + 3x gather/scatter conv) on 8 TRN2 NeuronCores. v2

Changes vs baseline:
  * bf16 message path: the per-layer feature table is stored COMPACT bf16
    [Np, 64ch] (128B rows); dma_gather descriptors read 256B = a PAIR of
    rows {2k, 2k+1}; idx = src>>1 (fits int16 without list splitting); an
    even-S / odd-S pair of one-hot matmuls selects the correct half.
  * no fp32 hi/lo split (PSUM still accumulates f32): removes the scalar
    hi-copy and vector lo-subtract from the critical path.
  * AllGather moves 6.4MB instead of 12.85MB.
  * S one-hot generation uses bf16 iota/drel (DVE 2x mode eligible), with
    even/odd interleaved in one tile -> one is_equal per gather batch.
  * dma_gather uses prepare_only on rotating queues + trigger_dma so Q7
    descriptor generation runs ahead of data dependencies and the 4 SWDGE
    queues' drains can overlap.
"""
import math
import os
import sys
import types

import numpy as np
import ml_dtypes

for _p in ("/opt/trn_rl_repo",):
    if _p not in sys.path and os.path.isdir(_p):
        sys.path.insert(0, _p)

import concourse.bacc as bacc
import concourse.bass as bass
import concourse.mybir as mybir
from concourse import tile
from concourse.bass_utils import run_bass_kernel_spmd

F32 = mybir.dt.float32
BF = mybir.dt.bfloat16

NCORES = 8
W = 128         # window rows (one S matrix column block)
NB = 32         # chunks per dma_gather batch
CH = 64         # hidden channels

LAST_PERF = None


def _install_ntff_hook():
    if "antenv.axon_hooks" in sys.modules:
        return
    try:
        from trn_agent_boot.trn_boot import _ntff_profile_via_ctypes

        mod = types.ModuleType("antenv.axon_hooks")
        box = [None]
        mod.set_axon_ntff_profile_hook = lambda h: box.__setitem__(0, h)
        mod.get_axon_ntff_profile_hook = lambda: box[0]
        mod.set_axon_ntff_profile_hook(
            _ntff_profile_via_ctypes("/opt/axon/libaxon_pjrt.so")
        )
        sys.modules["antenv.axon_hooks"] = mod
    except Exception:
        pass


def _prep(x, edge_index):
    """Host-side graph preprocessing. Returns (meta, per_core_inputs, newid)."""
    N = x.shape[0]
    E = edge_index.shape[1]
    NPC = N // NCORES
    NW = -(-NPC // W)            # windows per core
    NPCp = NW * W
    Np = NPCp * NCORES
    NT = NW

    src = np.ascontiguousarray(edge_index[0]).astype(np.int64)
    dst = np.ascontiguousarray(edge_index[1]).astype(np.int64)

    deg = 1.0 + np.bincount(dst, minlength=N).astype(np.float64)
    dinv = (1.0 / np.sqrt(deg)).astype(np.float32)

    # degree-sorted round-robin deal: rank r -> core (r//W)%8, window r//(W*8)
    order = np.argsort(-deg, kind="stable")
    r = np.arange(N)
    new_of_rank = ((r // W) % NCORES) * NPCp + (r // (W * NCORES)) * W + (r % W)
    newid = np.empty(N, np.int64)
    newid[order] = new_of_rank

    assert Np // 2 < 32768  # pair index must fit int16

    s_new = newid[src]
    d_new = newid[dst]
    core = d_new // NPCp
    dloc = d_new % NPCp
    win = dloc // W
    rel = (dloc % W).astype(np.int16)
    pidx = (s_new >> 1).astype(np.int16)   # pair index
    par = (s_new & 1).astype(np.int16)     # 0 -> even half, 1 -> odd half

    # per (core, win) counts -> shared chunk schedule (max over cores)
    cnt = np.zeros((NCORES, NW), np.int64)
    np.add.at(cnt, (core, win), 1)
    chmax = -(-cnt.max(axis=0) // 128)           # [NW] chunks per window
    base = np.cumsum(chmax) - chmax              # [NW] first chunk of window
    Ctot = int(chmax.sum())

    # slot assignment: edge -> (core, slot); vectorized cumcount per (core,win)
    key = core * NW + win
    o = np.argsort(key, kind="stable")
    ks = key[o]
    new_grp = np.empty(E, np.bool_)
    new_grp[0] = True
    new_grp[1:] = ks[1:] != ks[:-1]
    starts = np.nonzero(new_grp)[0]
    grp_of = np.cumsum(new_grp) - 1
    cumcount = np.arange(E) - starts[grp_of]
    slot_sorted = base[win[o]] * 128 + cumcount

    gidx = np.zeros((NCORES, Ctot * 128), np.int16)
    drelE = np.full((NCORES, Ctot * 128), W, np.int16)
    drelO = np.full((NCORES, Ctot * 128), W, np.int16)
    co, so = core[o], slot_sorted
    gidx[co, so] = pidx[o]
    mE = par[o] == 0
    drelE[co[mE], so[mE]] = rel[o][mE]
    mO = ~mE
    drelO[co[mO], so[mO]] = rel[o][mO]

    def pack_idx(a):  # [C*128] -> [128, C*8], idx i at [i%16, i//16], repl x8
        half = a.reshape(-1, 16).T
        return np.tile(half, (8, 1)).astype(np.int16)

    def f32_to_bf16_np(a):
        a = np.asarray(a, np.float32)
        u = a.view(np.uint32)
        # round-to-nearest-even
        u = (u + 0x7FFF + ((u >> 16) & 1)) >> 16
        return u.astype(np.uint16).view(ml_dtypes.bfloat16)

    def pack_drel(e_arr, o_arr, C):
        # [C*128] x2 -> [128, C, 2] bf16-as-uint16 (even, odd interleaved)
        e2 = e_arr.reshape(C, 128).T.astype(np.float32)   # [128, C]
        o2 = o_arr.reshape(C, 128).T.astype(np.float32)
        out = np.empty((128, C, 2), np.float32)
        out[:, :, 0] = e2
        out[:, :, 1] = o2
        return f32_to_bf16_np(out)

    dinv_new = np.zeros(Np, np.float32)
    dinv_new[newid] = dinv
    x_new = np.zeros((Np, x.shape[1]), np.float32)
    x_new[newid] = x

    per_core = []
    for c in range(NCORES):
        d = {}
        d["idx"] = pack_idx(gidx[c])
        d["drel"] = pack_drel(drelE[c], drelO[c], Ctot)
        d["dinv"] = np.ascontiguousarray(
            dinv_new[c * NPCp : (c + 1) * NPCp].reshape(NT, 128).T
        )
        d["xT"] = np.ascontiguousarray(x_new[c * NPCp : (c + 1) * NPCp].T)
        per_core.append(d)

    meta = dict(
        N=N, Np=Np, NPC=NPC, NPCp=NPCp, NW=NW, NT=NT,
        chmax=chmax, base=base, Ctot=Ctot, INC=x.shape[1],
    )
    return meta, per_core, newid


def _build(nc, meta, has_b1, has_b2, has_b3):
    """Trace the SPMD tile program for one core."""
    NT, NW, Np, NPCp = meta["NT"], meta["NW"], meta["Np"], meta["NPCp"]
    Ctot, chmax, base = meta["Ctot"], meta["chmax"], meta["base"]
    INC = meta["INC"]
    OUTC = 16
    NBATCH = -(-Ctot // NB)

    # ---- I/O -----------------------------------------------------------
    xT_d = nc.dram_tensor("xT", [INC, NPCp], F32, kind="ExternalInput")
    idx_d = nc.dram_tensor("idx", [128, Ctot * 8], mybir.dt.int16,
                           kind="ExternalInput")
    drel_d = nc.dram_tensor("drel", [128, Ctot * 2], BF, kind="ExternalInput")
    dinv_d = nc.dram_tensor("dinv", [128, NT], F32, kind="ExternalInput")
    w1_d = nc.dram_tensor("w1", [INC, CH], F32, kind="ExternalInput")
    w2_d = nc.dram_tensor("w2", [CH, CH], F32, kind="ExternalInput")
    w3_d = nc.dram_tensor("w3", [CH, OUTC], F32, kind="ExternalInput")
    identb_d = nc.dram_tensor("identb", [128, 128], BF, kind="ExternalInput")
    ident_d = nc.dram_tensor("ident", [128, 128], F32, kind="ExternalInput")
    iota_d = nc.dram_tensor("iota", [128, W], BF, kind="ExternalInput")
    b_d = {}
    if has_b1:
        b_d[1] = nc.dram_tensor("b1b", [128, CH], F32, kind="ExternalInput")
    if has_b2:
        b_d[2] = nc.dram_tensor("b2b", [128, CH], F32, kind="ExternalInput")
    if has_b3:
        b_d[3] = nc.dram_tensor("b3b", [128, OUTC], F32, kind="ExternalInput")
    out_d = nc.dram_tensor("out", [128, NT * OUTC], F32, kind="ExternalOutput")

    with tile.TileContext(nc) as tc:
        with (
            tc.tile_pool(name="const", bufs=1) as cpool,
            tc.tile_pool(name="hp", bufs=2) as hp_pool,
            tc.tile_pool(name="act", bufs=2) as act_pool,
            tc.tile_pool(name="xt", bufs=2) as xt_pool,
            tc.tile_pool(name="stage", bufs=4) as stg_pool,
            tc.tile_pool(name="msg", bufs=6) as msg_pool,
            tc.tile_pool(name="sgen", bufs=4) as s_pool,
            tc.tile_pool(name="aggps", bufs=4, space="PSUM") as agg_psum,
            tc.tile_pool(name="trps", bufs=2, space="PSUM") as tr_psum,
            tc.tile_pool(name="trxt", bufs=2, space="PSUM") as xt_psum,
            tc.tile_pool(name="dram", bufs=1, space="DRAM") as dram,
        ):
            # ---- residents ------------------------------------------------
            def load(shape, dtype, src):
                t = cpool.tile(shape, dtype, tag=f"c_{src.name}")
                nc.sync.dma_start(t[:], src[:])
                return t

            t_xT = load([INC, NPCp], F32, xT_d)
            t_idx = load([128, Ctot * 8], mybir.dt.int16, idx_d)
            t_drel = load([128, Ctot * 2], BF, drel_d)
            t_dinv = load([128, NT], F32, dinv_d)
            t_w1 = load([INC, CH], F32, w1_d)
            t_w2 = load([CH, CH], F32, w2_d)
            t_w3 = load([CH, OUTC], F32, w3_d)
            t_idb = load([128, 128], BF, identb_d)
            t_id = load([128, 128], F32, ident_d)
            t_iota = load([128, W], BF, iota_d)
            t_b = {k: load(v.shape, F32, v) for k, v in b_d.items()}

            def dinv_ap(t):
                return t_dinv[:][:, t : t + 1]

            ag_bufs = []
            for i in range(3):
                agi = dram.tile([NPCp, CH], BF, tag=f"ag_in{i}",
                                name=f"ag_in{i}")
                agf = dram.tile([Np, CH], BF, addr_space="Shared",
                                tag=f"ag_full{i}", name=f"ag_full{i}")
                ag_bufs.append((agi, agf))

            # ---- helpers --------------------------------------------------
            def allgather(i):
                ag_in, ag_full = ag_bufs[i]
                nc.gpsimd.collective_compute(
                    "AllGather",
                    mybir.AluOpType.bypass,
                    replica_groups=[list(range(NCORES))],
                    ins=[ag_in[:].opt()],
                    outs=[ag_full[:].opt()],
                )

            def store_hp(hp, i):
                # SBUF [128, NT*CH] bf16 -> DRAM row-major [NPCp, CH]
                dst = ag_bufs[i][0][:].rearrange("(t p) c -> p t c", p=128)
                src = hp[:].rearrange("p (t c) -> p t c", c=CH)
                with nc.allow_non_contiguous_dma("row-major store"):
                    nc.sync.dma_start(dst, src)

            PREP = os.environ.get("K_PREP", "0") == "1"
            PF = 4  # prep-ahead depth (one per SWDGE queue)

            def agg_prefetch(i, state):
                """Issue the first PF preps of layer i (before its AllGather)."""
                ag_full = ag_bufs[i][1]
                # pair-row view: [Np//2, 2*CH] bf16, 256B rows
                state["src_ap"] = ag_full[:].rearrange(
                    "(q two) c -> q (two c)", two=2
                )
                state["mtiles"] = []
                state["stiles"] = []
                state["prepped"] = 0
                state["done"] = 0
                if not PREP:
                    return
                for b in range(min(PF, NBATCH)):
                    _issue_prep(i, state, b)

            def _issue_prep(i, state, b):
                nb = min(NB, Ctot - b * NB)
                q = b % 4
                mt = msg_pool.tile([128, NB, 2 * CH], BF, tag="msg")
                kw = {}
                if PREP and b < PF:
                    kw = dict(prepare_only=True,
                              sem=nc.alloc_semaphore(f"sw_{i}_{b}"))
                nc.gpsimd.dma_gather(
                    mt[:][:, :nb, :],
                    state["src_ap"],
                    t_idx[:][:, b * NB * 8 : (b * NB + nb) * 8],
                    num_idxs=nb * 128,
                    num_idxs_reg=nb * 128,
                    elem_size=2 * CH,
                    elem_step=2 * CH,
                    single_packet=False,
                    queue_num=q,
                    **kw,
                )
                st = s_pool.tile([128, NB, 2, W], BF, tag="sg")
                nc.vector.tensor_tensor(
                    st[:][:, :nb, :, :],
                    t_drel[:].rearrange("p (c two) -> p c two", two=2)
                    [:, b * NB : b * NB + nb, :, None]
                    .broadcast_to([128, nb, 2, W]),
                    t_iota[:][:, None, None, :]
                    .broadcast_to([128, nb, 2, W]),
                    mybir.AluOpType.is_equal,
                )
                state["mtiles"].append(mt)
                state["stiles"].append(st)
                state["prepped"] += 1

            def agg_layer(i, hp_tile, evict, state):
                """Gather + segment-sum for one layer (even/odd pair select)."""
                mtiles = state["mtiles"]
                stiles = state["stiles"]

                def ensure(j):
                    while j >= state["done"] * NB:
                        b = state["done"]
                        if PREP and b < min(PF, NBATCH):
                            nc.gpsimd.trigger_dma(count=None, queue_num=b % 4)
                        else:
                            _issue_prep(i, state, b)
                        state["done"] += 1

                for t in range(NT):
                    chunks = list(range(int(base[t]), int(base[t] + chmax[t])))
                    ps = agg_psum.tile([128, CH], F32, tag="aggps")
                    nc.tensor.matmul(
                        ps[:],
                        t_idb[:],
                        hp_tile[:][:, t * CH : (t + 1) * CH],
                        start=True,
                        stop=(not chunks),
                    )
                    nmm = len(chunks) * 2
                    k = 0
                    for j in chunks:
                        ensure(j)
                        b, loc = divmod(j, NB)
                        for half in (0, 1):
                            k += 1
                            nc.tensor.matmul(
                                ps[:],
                                stiles[b][:][:, loc, half, :],
                                mtiles[b][:][:, loc,
                                             half * CH : (half + 1) * CH],
                                start=False,
                                stop=(k == nmm),
                            )
                    evict(t, ps)

            # ---- layer 1: transform x @ W1 -------------------------------
            hp1 = hp_pool.tile([128, NT * CH], BF, tag="hp")
            for t in range(NT):
                ps = tr_psum.tile([128, CH], F32, tag="trps")
                nc.tensor.matmul(
                    ps[:],
                    t_xT[:][:, t * 128 : (t + 1) * 128],
                    t_w1[:],
                    start=True,
                    stop=True,
                )
                nc.scalar.activation(
                    hp1[:][:, t * CH : (t + 1) * CH],
                    ps[:],
                    mybir.ActivationFunctionType.Copy,
                    bias=0.0,
                    scale=dinv_ap(t),
                )
            store_hp(hp1, 0)
            st1 = {}
            agg_prefetch(0, st1)
            allgather(0)

            # ---- layer 1 aggregation + lrelu ------------------------------
            act1 = act_pool.tile([128, NT * CH], F32, tag="act")

            def evict_lrelu(act_tile, has_b, bkey, then=None):
                def _e(t, ps):
                    stg = stg_pool.tile([128, CH], F32, tag="stg")
                    if has_b:
                        nc.vector.scalar_tensor_tensor(
                            stg[:],
                            ps[:],
                            dinv_ap(t),
                            t_b[bkey][:],
                            mybir.AluOpType.mult,
                            mybir.AluOpType.add,
                        )
                    else:
                        nc.scalar.activation(
                            stg[:],
                            ps[:],
                            mybir.ActivationFunctionType.Copy,
                            bias=0.0,
                            scale=dinv_ap(t),
                        )
                    nc.vector.scalar_tensor_tensor(
                        act_tile[:][:, t * CH : (t + 1) * CH],
                        stg[:],
                        0.2,
                        stg[:],
                        mybir.AluOpType.mult,
                        mybir.AluOpType.max,
                    )
                    if then is not None:
                        then(t, act_tile[:][:, t * CH : (t + 1) * CH])
                return _e

            def transform_w(w_tile, hp_tile):
                """Inline per-window transform: hp[:, t] = (act_t @ W) * dinv."""
                def _t(t, act_ap):
                    psx = xt_psum.tile([CH, 128], F32, tag="trxt")
                    nc.tensor.transpose(psx[:], act_ap, t_id[:])
                    xt = xt_pool.tile([CH, 128], F32, tag="xt")
                    nc.scalar.copy(xt[:], psx[:])
                    ps = tr_psum.tile([128, CH], F32, tag="trps")
                    nc.tensor.matmul(ps[:], xt[:], w_tile[:],
                                     start=True, stop=True)
                    nc.scalar.activation(
                        hp_tile[:][:, t * CH : (t + 1) * CH],
                        ps[:],
                        mybir.ActivationFunctionType.Copy,
                        bias=0.0,
                        scale=dinv_ap(t),
                    )
                return _t

            hp2 = hp_pool.tile([128, NT * CH], BF, tag="hp")
            agg_layer(0, hp1,
                      evict_lrelu(act1, has_b1, 1, then=transform_w(t_w2, hp2)),
                      st1)
            store_hp(hp2, 1)
            st2 = {}
            agg_prefetch(1, st2)
            allgather(1)

            # ---- layer 2 aggregation + lrelu ------------------------------
            act2 = act_pool.tile([128, NT * CH], F32, tag="act")
            hp3 = hp_pool.tile([128, NT * CH], BF, tag="hp")

            def prescale3(t, act_ap):
                nc.scalar.activation(
                    hp3[:][:, t * CH : (t + 1) * CH],
                    act_ap,
                    mybir.ActivationFunctionType.Copy,
                    bias=0.0,
                    scale=dinv_ap(t),
                )

            agg_layer(1, hp2, evict_lrelu(act2, has_b2, 2, then=prescale3),
                      st2)
            store_hp(hp3, 2)
            st3 = {}
            agg_prefetch(2, st3)
            allgather(2)

            # ---- layer 3 aggregation (scale only) -------------------------
            agg3 = act_pool.tile([128, NT * CH], F32, tag="act")

            def evict_scale(t, ps):
                nc.scalar.activation(
                    agg3[:][:, t * CH : (t + 1) * CH],
                    ps[:],
                    mybir.ActivationFunctionType.Copy,
                    bias=0.0,
                    scale=dinv_ap(t),
                )

            outsb = cpool.tile([128, NT * OUTC], F32, tag="outsb")

            def evict3(t, ps):
                evict_scale(t, ps)
                psx = xt_psum.tile([CH, 128], F32, tag="trxt")
                nc.tensor.transpose(
                    psx[:], agg3[:][:, t * CH : (t + 1) * CH], t_id[:]
                )
                xt = xt_pool.tile([CH, 128], F32, tag="xt")
                nc.scalar.copy(xt[:], psx[:])
                ps2 = tr_psum.tile([128, OUTC], F32, tag="trps")
                nc.tensor.matmul(ps2[:], xt[:], t_w3[:], start=True, stop=True)
                o_ap = outsb[:][:, t * OUTC : (t + 1) * OUTC]
                if has_b3:
                    stg = stg_pool.tile([128, OUTC], F32, tag="stgo")
                    nc.vector.tensor_add(stg[:], ps2[:], t_b[3][:])
                    nc.scalar.activation(
                        o_ap, stg[:], mybir.ActivationFunctionType.Tanh
                    )
                else:
                    nc.scalar.activation(
                        o_ap, ps2[:], mybir.ActivationFunctionType.Tanh
                    )

            agg_layer(2, hp3, evict3, st3)
            nc.sync.dma_start(out_d[:], outsb[:])

    nc.finalize()


def kernel(x, edge_index, W1, b1, W2, b2, W3, b3):
    global LAST_PERF
    x = np.asarray(x, np.float32)
    edge_index = np.asarray(edge_index)
    W1 = np.asarray(W1, np.float32)
    W2 = np.asarray(W2, np.float32)
    W3 = np.asarray(W3, np.float32)
    b1 = np.asarray(b1, np.float32)
    b2 = np.asarray(b2, np.float32)
    b3 = np.asarray(b3, np.float32)

    meta, per_core, newid = _prep(x, edge_index)
    has_b1 = bool(np.any(b1))
    has_b2 = bool(np.any(b2))
    has_b3 = bool(np.any(b3))

    if os.environ.get("BASS_TRACE"):
        _install_ntff_hook()

    nc = bacc.Bacc("TRN2", target_bir_lowering=False, debug=False,
                   num_devices=NCORES, num_swdge_queues=4)
    _build(nc, meta, has_b1, has_b2, has_b3)

    NT = meta["NT"]

    def f32_to_bf16_np(a):
        a = np.asarray(a, np.float32)
        u = a.view(np.uint32)
        u = (u + 0x7FFF + ((u >> 16) & 1)) >> 16
        return u.astype(np.uint16).view(ml_dtypes.bfloat16)

    iota = f32_to_bf16_np(
        np.broadcast_to(np.arange(W, dtype=np.float32), (128, W)).copy()
    )
    ident = np.eye(128, dtype=np.float32)
    identb = f32_to_bf16_np(ident)
    common = {
        "w1": W1, "w2": W2, "w3": W3, "ident": ident, "identb": identb,
        "iota": iota,
    }
    if has_b1:
        common["b1b"] = np.broadcast_to(b1, (128, 64)).copy()
    if has_b2:
        common["b2b"] = np.broadcast_to(b2, (128, 64)).copy()
    if has_b3:
        common["b3b"] = np.broadcast_to(b3, (128, 16)).copy()

    in_maps = [{**per_core[c], **common} for c in range(NCORES)]
    res = run_bass_kernel_spmd(nc, in_maps, core_ids=list(range(NCORES)))
    LAST_PERF = res

    N = meta["N"]
    NPCp = meta["NPCp"]
    full = np.empty((meta["Np"], 16), np.float32)
    for c in range(NCORES):
        o = res.results[c]["out"]  # [128, NT*16]
        full[c * NPCp : (c + 1) * NPCp] = (
            o.reshape(128, NT, 16).transpose(1, 0, 2).reshape(NPCp, 16)
        )
    out = np.empty((N, 16), np.float32)
    out[:] = full[newid]
    return out
